# revision 5
# baseline (speedup 1.0000x reference)
"""HGNN conv on 8 TRN2 NeuronCores.

out = Dv^-1/2 H De^-1 H^T Dv^-1/2 X W + b
  X[20000,128] f32, H[20000,4096] int32 (0/1), weight[128,128], bias[128]

Strategy: shard N (nodes) row-wise across 8 cores (2500 rows each).
Per core, H's shard (41MB int32) is read from HBM exactly once:
  - pass A: stream 128-row bands, cast int32->bf16 (ACT), row-reduce for
    v_deg (DVE), mm1 accumulates T^T = (Dv^-1/2 X)^T-style partial in PSUM
    with Y=dv*X stationary and H moving;
  - each band is also xbar-DMA-transposed (2-byte path) into e-major strips
    and quantized to fp8e4 (exact for 0/1) for a 10.3MB resident H^T;
  - e_deg comes from free-axis reduces of the transposed strips.
One packed AllReduce carries T^T partial [128,4096] + e_deg [128,32] (f32).
Then T2 = De^-1 * T via PE transpose + ACT scale, mm2 = T2^T @ H^T with
bf16 stationary x fp8 moving, and out = dv * (Z @ W) + b.
"""

import numpy as np
import sys

sys.path.insert(0, "/opt/trn_rl_repo")

from concourse import bass, bacc, tile, mybir  # noqa: E402
from concourse.bass_utils import run_bass_kernel_spmd  # noqa: E402

FP32 = mybir.dt.float32
BF16 = mybir.dt.bfloat16
FP8 = mybir.dt.float8e4
I32 = mybir.dt.int32

Copy = mybir.ActivationFunctionType.Copy
AX = mybir.AxisListType
ALU = mybir.AluOpType

N_CORES = 8
N, E, F = 20000, 4096, 128
NSH = N // N_CORES            # 2500 rows per core
NB = 20                       # bands: 19 full + 1 partial
LAST_ROWS = NSH - (NB - 1) * 128   # 68
LAST_PAD = 80                 # xbar needs partition %16==0
NCOLS = (NB - 1) * 128 + LAST_PAD  # 2512 strip columns
EB = E // 128                 # 32 e-blocks
AR_COLS = E + EB              # 4128: T^T columns + packed e_deg

_CACHE = {}


def _build_nc():
    nc = bacc.Bacc(
        "TRN2",
        target_bir_lowering=False,
        debug=False,
        enable_asserts=False,
        num_devices=N_CORES,
    )
    X_d = nc.dram_tensor("X", [NSH, F], FP32, kind="ExternalInput")
    H_d = nc.dram_tensor("H", [NSH, E], I32, kind="ExternalInput")
    W_d = nc.dram_tensor("weight", [F, F], FP32, kind="ExternalInput")
    B_d = nc.dram_tensor("bias", [1, F], FP32, kind="ExternalInput")
    I_d = nc.dram_tensor("identity", [128, 128], FP32, kind="ExternalInput")
    O_d = nc.dram_tensor("out", [NSH, F], FP32, kind="ExternalOutput")

    rg = [list(range(N_CORES))]

    with tile.TileContext(nc) as tc:
        with (
            tc.tile_pool(name="const", bufs=1) as constp,
            tc.tile_pool(name="res", bufs=1) as resp,
            tc.tile_pool(name="h32", bufs=3) as h32p,
            tc.tile_pool(name="hbf", bufs=2) as hbfp,
            tc.tile_pool(name="htr", bufs=2) as htrp,
            tc.tile_pool(name="xs", bufs=2) as xsp,
            tc.tile_pool(name="y", bufs=2) as yp,
            tc.tile_pool(name="ost", bufs=2) as ostp,
            tc.tile_pool(name="psum", bufs=8, space="PSUM") as psump,
            tc.tile_pool(name="dram", bufs=1, space="DRAM") as dramp,
        ):
            # ---- constants ----
            ident = constp.tile([128, 128], FP32)
            nc.sync.dma_start(ident[:], I_d[:])
            wstage = constp.tile([128, 128], FP32)
            nc.sync.dma_start(wstage[:], W_d[:])
            Wb = constp.tile([128, 128], BF16)
            nc.scalar.copy(Wb[:], wstage[:])
            bstage = constp.tile([1, 128], FP32)
            nc.sync.dma_start(bstage[:], B_d[:])
            bias_bc = constp.tile([128, 128], FP32)
            nc.gpsimd.partition_broadcast(bias_bc[:], bstage[:], channels=128)

            # ---- resident ----
            strips = resp.tile([128, EB, NCOLS], FP8)   # H^T: strip g, part p <-> e=g*128+p
            dv = resp.tile([128, NB], FP32)             # dv_inv_sqrt, col per band
            edp = resp.tile([128, NB * 32], FP32)       # e_deg partials, col=(2nb+h)*16+g16
            T2 = resp.tile([128, E], BF16)              # de_inv * T, e-major tiles
            dei = resp.tile([128, EB], FP32)
            zt = resp.tile([128, NSH], BF16)            # Z^T

            tacc = [psump.tile([128, 512], FP32, tag="ps", name=f"tacc{k}") for k in range(8)]

            # ================ pass A ================
            for nb in range(NB):
                rows = 128 if nb < NB - 1 else LAST_ROWS
                padr = 128 if nb < NB - 1 else LAST_PAD
                r0 = nb * 128

                hbf = hbfp.tile([128, E], BF16, tag="hbf")
                if nb == NB - 1:
                    # zero pad rows (partition slices must be 32-aligned,
                    # so clear the whole tile before the partial-row cast)
                    nc.vector.memset(hbf[:, :], 0.0)
                for h in range(2):
                    h32 = h32p.tile([128, 2048], I32, tag="h32")
                    nc.sync.dma_start(
                        h32[:rows, :], H_d[r0 : r0 + rows, h * 2048 : (h + 1) * 2048]
                    )
                    nc.scalar.copy(hbf[:rows, h * 2048 : (h + 1) * 2048], h32[:rows, :])

                # v_deg -> dv_inv_sqrt column
                nc.vector.tensor_reduce(
                    dv[:rows, nb : nb + 1], hbf[:rows, :], axis=AX.X, op=ALU.add
                )
                nc.vector.tensor_scalar_max(
                    dv[:rows, nb : nb + 1], dv[:rows, nb : nb + 1], 1.0
                )
                nc.scalar.sqrt(dv[:rows, nb : nb + 1], dv[:rows, nb : nb + 1])
                nc.vector.reciprocal(dv[:rows, nb : nb + 1], dv[:rows, nb : nb + 1])

                # Y = dv * X  (bf16)
                xs = xsp.tile([128, F], FP32, tag="xs")
                nc.sync.dma_start(xs[:rows, :], X_d[r0 : r0 + rows, :])
                y = yp.tile([128, F], BF16, tag="y")
                nc.scalar.activation(
                    y[:rows, :], xs[:rows, :], Copy, scale=dv[:rows, nb : nb + 1]
                )

                # mm1: T^T[f, e] += Y^T H, 8 psum banks of 512 e-cols
                for k in range(8):
                    nc.tensor.matmul(
                        tacc[k][:, :],
                        y[:rows, :],
                        hbf[:rows, k * 512 : (k + 1) * 512],
                        start=(nb == 0),
                        stop=(nb == NB - 1),
                    )

                # xbar transpose -> e-major, e_deg partial, fp8 store
                for h in range(2):
                    htr = htrp.tile([128, 16, 128], BF16, tag="htr")
                    nc.sync.dma_start_transpose(
                        htr[:, :, :padr], hbf[:padr, h * 2048 : (h + 1) * 2048]
                    )
                    nc.vector.tensor_reduce(
                        edp[:, (2 * nb + h) * 16 : (2 * nb + h + 1) * 16],
                        htr[:, :, :padr],
                        axis=AX.X,
                        op=ALU.add,
                    )
                    nc.gpsimd.tensor_copy(
                        strips[:, h * 16 : (h + 1) * 16, r0 : r0 + padr],
                        htr[:, :, :padr],
                    )

            # ================ AllReduce ================
            tpre = resp.tile([128, AR_COLS], FP32, tag="tbuf")
            for k in range(8):
                nc.scalar.copy(tpre[:, k * 512 : (k + 1) * 512], tacc[k][:, :])
            # e_deg partial: sum band partials; edp col=(band2)*16+g16, strip g=bh*16+g16
            # view [128, (b2 g)] -> [128, g16? ] ; col = b2*16+g16 with b2=2nb+h
            # strip index g = h*16+g16 ; col = nb*32 + h*16 + g16 = nb*32 + g
            nc.vector.tensor_reduce(
                tpre[:, E : E + EB],
                edp[:].rearrange("p (b g) -> p g b", g=EB),
                axis=AX.X,
                op=ALU.add,
            )
            ar_in = dramp.tile([128, AR_COLS], FP32, tag="arin")
            ar_out = dramp.tile([128, AR_COLS], FP32, tag="arout", addr_space="Shared")
            nc.sync.dma_start(ar_in[:], tpre[:])
            nc.gpsimd.collective_compute(
                "AllReduce",
                ALU.add,
                replica_groups=rg,
                ins=[ar_in[:].opt()],
                outs=[ar_out[:].opt()],
            )
            tpost = resp.tile([128, AR_COLS], FP32, tag="tbuf")
            nc.sync.dma_start(tpost[:], ar_out[:])

            # de_inv
            nc.vector.tensor_scalar_max(dei[:], tpost[:, E : E + EB], 1.0)
            nc.vector.reciprocal(dei[:], dei[:])

            # T2[e,f] = de_inv[e] * T[e,f]  (PE transpose of T^T tiles)
            for g in range(EB):
                ptr = psump.tile([128, 512], FP32, tag="ps", name="ptr")
                nc.tensor.transpose(
                    ptr[:, :128], tpost[:, g * 128 : (g + 1) * 128], ident[:]
                )
                nc.scalar.activation(
                    T2[:, g * 128 : (g + 1) * 128], ptr[:, :128], Copy,
                    scale=dei[:, g : g + 1],
                )

            # mm2: Z^T[f, n] = sum_e T2[e,f] * H^T[e,n]
            zchunks = [(0, 512), (512, 512), (1024, 512), (1536, 512), (2048, NCOLS - 2048)]
            pzt = [psump.tile([128, 512], FP32, tag="ps", name=f"pz{k}") for k in range(5)]
            for g in range(EB):
                for ci, (c0, cl) in enumerate(zchunks):
                    nc.tensor.matmul(
                        pzt[ci][:, :cl],
                        T2[:, g * 128 : (g + 1) * 128],
                        strips[:, g : g + 1, c0 : c0 + cl],
                        start=(g == 0),
                        stop=(g == EB - 1),
                    )
            for ci, (c0, cl) in enumerate(zchunks):
                cl2 = min(c0 + cl, NSH) - c0
                nc.scalar.copy(zt[:, c0 : c0 + cl2], pzt[ci][:, :cl2])

            # final: out[n,:] = dv[n] * (Z @ W) + b
            for nb in range(NB):
                rows = 128 if nb < NB - 1 else LAST_ROWS
                r0 = nb * 128
                po = psump.tile([128, 512], FP32, tag="ps", name="po")
                nc.tensor.matmul(
                    po[:rows, :128], zt[:, r0 : r0 + rows], Wb[:], start=True, stop=True
                )
                ost = ostp.tile([128, 128], FP32, tag="ost")
                nc.scalar.activation(
                    ost[:rows, :], po[:rows, :128], Copy, scale=dv[:rows, nb : nb + 1]
                )
                nc.vector.tensor_tensor(
                    ost[:rows, :], ost[:rows, :], bias_bc[:rows, :], op=ALU.add
                )
                nc.sync.dma_start(O_d[r0 : r0 + rows, :], ost[:rows, :])

    nc.compile()
    return nc


def _get_nc():
    if "nc" not in _CACHE:
        _CACHE["nc"] = _build_nc()
    return _CACHE["nc"]


def _in_maps(X, H, weight, bias):
    X = np.ascontiguousarray(X, dtype=np.float32)
    H = np.ascontiguousarray(H, dtype=np.int32)
    w = np.ascontiguousarray(weight, dtype=np.float32)
    b = np.ascontiguousarray(bias, dtype=np.float32).reshape(1, F)
    ident = np.eye(128, dtype=np.float32)
    maps = []
    for i in range(N_CORES):
        maps.append(
            {
                "X": X[i * NSH : (i + 1) * NSH],
                "H": H[i * NSH : (i + 1) * NSH],
                "weight": w,
                "bias": b,
                "identity": ident,
            }
        )
    return maps


def _run(in_maps, trace=False, **kw):
    nc = _get_nc()
    return run_bass_kernel_spmd(
        nc, in_maps, core_ids=list(range(N_CORES)), trace=trace, **kw
    )


def kernel(X, H, weight, bias, **_unused):
    res = _run(_in_maps(X, H, weight, bias))
    return np.concatenate(
        [res.results[i]["out"] for i in range(N_CORES)], axis=0
    ).astype(np.float32)


# revision 7
# speedup vs baseline: 1.0097x; 1.0097x over previous
"""HGNN conv on 8 TRN2 NeuronCores.

out = Dv^-1/2 H De^-1 H^T Dv^-1/2 X W + b
  X[20000,128] f32, H[20000,4096] int32 (0/1), weight[128,128], bias[128]

Strategy: shard N (nodes) row-wise across 8 cores (2500 rows each).
Per core, H's shard (41MB int32) is read from HBM exactly once:
  - pass A: stream 128-row bands, cast int32->bf16 (ACT), row-reduce for
    v_deg (DVE), mm1 accumulates T^T = (Dv^-1/2 X)^T-style partial in PSUM
    with Y=dv*X stationary and H moving;
  - each band is also xbar-DMA-transposed (2-byte path) into e-major strips
    and quantized to fp8e4 (exact for 0/1) for a 10.3MB resident H^T;
  - e_deg comes from free-axis reduces of the transposed strips.
One packed AllReduce carries T^T partial [128,4096] + e_deg [128,32] (f32).
Then T2 = De^-1 * T via PE transpose + ACT scale, mm2 = T2^T @ H^T with
bf16 stationary x fp8 moving, and out = dv * (Z @ W) + b.
"""

import numpy as np
import sys

sys.path.insert(0, "/opt/trn_rl_repo")

from concourse import bass, bacc, tile, mybir  # noqa: E402
from concourse.bass_utils import run_bass_kernel_spmd  # noqa: E402

FP32 = mybir.dt.float32
BF16 = mybir.dt.bfloat16
FP8 = mybir.dt.float8e4
I32 = mybir.dt.int32

Copy = mybir.ActivationFunctionType.Copy
AX = mybir.AxisListType
ALU = mybir.AluOpType

N_CORES = 8
N, E, F = 20000, 4096, 128
NSH = N // N_CORES            # 2500 rows per core
NB = 20                       # bands: 19 full + 1 partial
LAST_ROWS = NSH - (NB - 1) * 128   # 68
LAST_PAD = 80                 # xbar needs partition %16==0
NCOLS = (NB - 1) * 128 + LAST_PAD  # 2512 strip columns
EB = E // 128                 # 32 e-blocks
AR_COLS = E + EB              # 4128: T^T columns + packed e_deg

_CACHE = {}


def _build_nc(ar_bf16=False):
    ARDT = BF16 if ar_bf16 else FP32
    nc = bacc.Bacc(
        "TRN2",
        target_bir_lowering=False,
        debug=False,
        enable_asserts=False,
        num_devices=N_CORES,
    )
    X_d = nc.dram_tensor("X", [NSH, F], FP32, kind="ExternalInput")
    H_d = nc.dram_tensor("H", [NSH, E], I32, kind="ExternalInput")
    W_d = nc.dram_tensor("weight", [F, F], FP32, kind="ExternalInput")
    B_d = nc.dram_tensor("bias", [1, F], FP32, kind="ExternalInput")
    I_d = nc.dram_tensor("identity", [128, 128], FP32, kind="ExternalInput")
    O_d = nc.dram_tensor("out", [NSH, F], FP32, kind="ExternalOutput")

    rg = [list(range(N_CORES))]

    with tile.TileContext(nc) as tc:
        with (
            tc.tile_pool(name="const", bufs=1) as constp,
            tc.tile_pool(name="res", bufs=1) as resp,
            tc.tile_pool(name="h32", bufs=3) as h32p,
            tc.tile_pool(name="hbf", bufs=2) as hbfp,
            tc.tile_pool(name="htr", bufs=2) as htrp,
            tc.tile_pool(name="xs", bufs=2) as xsp,
            tc.tile_pool(name="y", bufs=2) as yp,
            tc.tile_pool(name="ost", bufs=2) as ostp,
            tc.tile_pool(name="psum", bufs=8, space="PSUM") as psump,
            tc.tile_pool(name="dram", bufs=1, space="DRAM") as dramp,
        ):
            # ---- constants ----
            ident = constp.tile([128, 128], FP32)
            nc.sync.dma_start(ident[:], I_d[:])
            wstage = constp.tile([128, 128], FP32)
            nc.sync.dma_start(wstage[:], W_d[:])
            Wb = constp.tile([128, 128], BF16)
            nc.scalar.copy(Wb[:], wstage[:])
            bstage = constp.tile([1, 128], FP32)
            nc.sync.dma_start(bstage[:], B_d[:])
            bias_bc = constp.tile([128, 128], FP32)
            nc.gpsimd.partition_broadcast(bias_bc[:], bstage[:], channels=128)
            identb = constp.tile([128, 128], BF16)
            if ar_bf16:
                nc.scalar.copy(identb[:], ident[:])

            # ---- resident ----
            strips = resp.tile([128, EB, NCOLS], FP8)   # H^T: strip g, part p <-> e=g*128+p
            dv = resp.tile([128, NB], FP32)             # dv_inv_sqrt, col per band
            edp = resp.tile([128, NB * 32], FP32)       # e_deg partials, col=(2nb+h)*16+g16
            T2 = resp.tile([128, E], BF16)              # de_inv * T, e-major tiles
            dei = resp.tile([128, EB], FP32)
            zt = resp.tile([128, NSH], BF16)            # Z^T

            tacc = [psump.tile([128, 512], FP32, tag="ps", name=f"tacc{k}") for k in range(8)]

            # ================ pass A ================
            for nb in range(NB):
                rows = 128 if nb < NB - 1 else LAST_ROWS
                padr = 128 if nb < NB - 1 else LAST_PAD
                r0 = nb * 128

                hbf = hbfp.tile([128, E], BF16, tag="hbf")
                if nb == NB - 1:
                    # zero pad rows (partition slices must be 32-aligned,
                    # so clear the whole tile before the partial-row cast)
                    nc.vector.memset(hbf[:, :], 0.0)
                for h in range(2):
                    h32 = h32p.tile([128, 2048], I32, tag="h32")
                    nc.sync.dma_start(
                        h32[:rows, :], H_d[r0 : r0 + rows, h * 2048 : (h + 1) * 2048]
                    )
                    nc.scalar.copy(hbf[:rows, h * 2048 : (h + 1) * 2048], h32[:rows, :])

                # v_deg -> dv_inv_sqrt column
                nc.vector.tensor_reduce(
                    dv[:rows, nb : nb + 1], hbf[:rows, :], axis=AX.X, op=ALU.add
                )
                nc.vector.tensor_scalar_max(
                    dv[:rows, nb : nb + 1], dv[:rows, nb : nb + 1], 1.0
                )
                nc.scalar.sqrt(dv[:rows, nb : nb + 1], dv[:rows, nb : nb + 1])
                nc.vector.reciprocal(dv[:rows, nb : nb + 1], dv[:rows, nb : nb + 1])

                # Y = dv * X  (bf16)
                xs = xsp.tile([128, F], FP32, tag="xs")
                nc.sync.dma_start(xs[:rows, :], X_d[r0 : r0 + rows, :])
                y = yp.tile([128, F], BF16, tag="y")
                nc.scalar.activation(
                    y[:rows, :], xs[:rows, :], Copy, scale=dv[:rows, nb : nb + 1]
                )

                # mm1: T^T[f, e] += Y^T H, 8 psum banks of 512 e-cols
                for k in range(8):
                    nc.tensor.matmul(
                        tacc[k][:, :],
                        y[:rows, :],
                        hbf[:rows, k * 512 : (k + 1) * 512],
                        start=(nb == 0),
                        stop=(nb == NB - 1),
                    )

                # xbar transpose -> e-major, e_deg partial, fp8 store
                for h in range(2):
                    htr = htrp.tile([128, 16, 128], BF16, tag="htr")
                    nc.sync.dma_start_transpose(
                        htr[:, :, :padr], hbf[:padr, h * 2048 : (h + 1) * 2048]
                    )
                    nc.vector.tensor_reduce(
                        edp[:, (2 * nb + h) * 16 : (2 * nb + h + 1) * 16],
                        htr[:, :, :padr],
                        axis=AX.X,
                        op=ALU.add,
                    )
                    nc.gpsimd.tensor_copy(
                        strips[:, h * 16 : (h + 1) * 16, r0 : r0 + padr],
                        htr[:, :, :padr],
                    )

            # ================ AllReduce ================
            tpre = resp.tile([128, AR_COLS], ARDT, tag="tbuf")
            for k in range(8):
                nc.scalar.copy(tpre[:, k * 512 : (k + 1) * 512], tacc[k][:, :])
            # e_deg partial: sum band partials; edp col=(band2)*16+g16, strip g=bh*16+g16
            # view [128, (b2 g)] -> [128, g16? ] ; col = b2*16+g16 with b2=2nb+h
            # strip index g = h*16+g16 ; col = nb*32 + h*16 + g16 = nb*32 + g
            edf = resp.tile([128, EB], FP32, tag="edf")
            nc.vector.tensor_reduce(
                edf[:],
                edp[:].rearrange("p (b g) -> p g b", g=EB),
                axis=AX.X,
                op=ALU.add,
            )
            nc.scalar.copy(tpre[:, E : E + EB], edf[:])
            ar_in = dramp.tile([128, AR_COLS], ARDT, tag="arin")
            ar_out = dramp.tile([128, AR_COLS], ARDT, tag="arout", addr_space="Shared")
            nc.sync.dma_start(ar_in[:], tpre[:])
            nc.gpsimd.collective_compute(
                "AllReduce",
                ALU.add,
                replica_groups=rg,
                ins=[ar_in[:].opt()],
                outs=[ar_out[:].opt()],
            )
            tpost = resp.tile([128, AR_COLS], ARDT, tag="tbuf")
            nc.sync.dma_start(tpost[:], ar_out[:])

            # de_inv
            nc.vector.tensor_scalar_max(dei[:], tpost[:, E : E + EB], 1.0)
            nc.vector.reciprocal(dei[:], dei[:])

            # T2[e,f] = de_inv[e] * T[e,f]  (PE transpose of T^T tiles)
            for g in range(EB):
                ptr = psump.tile([128, 512], ARDT, tag="ps", name="ptr")
                nc.tensor.transpose(
                    ptr[:, :128],
                    tpost[:, g * 128 : (g + 1) * 128],
                    identb[:] if ar_bf16 else ident[:],
                )
                nc.scalar.activation(
                    T2[:, g * 128 : (g + 1) * 128], ptr[:, :128], Copy,
                    scale=dei[:, g : g + 1],
                )

            # mm2: Z^T[f, n] = sum_e T2[e,f] * H^T[e,n]
            zchunks = [(0, 512), (512, 512), (1024, 512), (1536, 512), (2048, NCOLS - 2048)]
            pzt = [psump.tile([128, 512], FP32, tag="ps", name=f"pz{k}") for k in range(5)]
            for g in range(EB):
                for ci, (c0, cl) in enumerate(zchunks):
                    nc.tensor.matmul(
                        pzt[ci][:, :cl],
                        T2[:, g * 128 : (g + 1) * 128],
                        strips[:, g : g + 1, c0 : c0 + cl],
                        start=(g == 0),
                        stop=(g == EB - 1),
                    )
            for ci, (c0, cl) in enumerate(zchunks):
                cl2 = min(c0 + cl, NSH) - c0
                nc.scalar.copy(zt[:, c0 : c0 + cl2], pzt[ci][:, :cl2])

            # final: out[n,:] = dv[n] * (Z @ W) + b
            for nb in range(NB):
                rows = 128 if nb < NB - 1 else LAST_ROWS
                r0 = nb * 128
                po = psump.tile([128, 512], FP32, tag="ps", name="po")
                nc.tensor.matmul(
                    po[:rows, :128], zt[:, r0 : r0 + rows], Wb[:], start=True, stop=True
                )
                ost = ostp.tile([128, 128], FP32, tag="ost")
                nc.scalar.activation(
                    ost[:rows, :], po[:rows, :128], Copy, scale=dv[:rows, nb : nb + 1]
                )
                nc.vector.tensor_tensor(
                    ost[:rows, :], ost[:rows, :], bias_bc[:rows, :], op=ALU.add
                )
                nc.sync.dma_start(O_d[r0 : r0 + rows, :], ost[:rows, :])

    nc.compile()
    return nc


AR_BF16 = True  # bf16 AllReduce: verified on HW, rel err 3.3e-03


def _get_nc():
    if "nc" not in _CACHE:
        _CACHE["nc"] = _build_nc(ar_bf16=AR_BF16)
    return _CACHE["nc"]


def _in_maps(X, H, weight, bias):
    X = np.ascontiguousarray(X, dtype=np.float32)
    H = np.ascontiguousarray(H, dtype=np.int32)
    w = np.ascontiguousarray(weight, dtype=np.float32)
    b = np.ascontiguousarray(bias, dtype=np.float32).reshape(1, F)
    ident = np.eye(128, dtype=np.float32)
    maps = []
    for i in range(N_CORES):
        maps.append(
            {
                "X": X[i * NSH : (i + 1) * NSH],
                "H": H[i * NSH : (i + 1) * NSH],
                "weight": w,
                "bias": b,
                "identity": ident,
            }
        )
    return maps


def _run(in_maps, trace=False, **kw):
    nc = _get_nc()
    return run_bass_kernel_spmd(
        nc, in_maps, core_ids=list(range(N_CORES)), trace=trace, **kw
    )


def kernel(X, H, weight, bias, **_unused):
    res = _run(_in_maps(X, H, weight, bias))
    return np.concatenate(
        [res.results[i]["out"] for i in range(N_CORES)], axis=0
    ).astype(np.float32)
